# revision 6
# baseline (speedup 1.0000x reference)
"""Trainium2 Bass kernel v2 for nn_MetricLoss — bf16-plane formulation.

Math (per row): S0 = sym2x2(pred), M = sym2x2(actual) SPD. The loss
||log(M0^{-1/2} M M0^{-1/2})||_F^2 with M0 = exp(S0) depends only on the
two generalized eigenvalues of (M, M0):
    loss_row = (L^2 + G^2)/2
    L  = ln det M - tr S0
    G  = 2 arccosh(k),  k = shat / (2 sqrt(det M))
    shat = cosh(r/2)(A+C) - sinh(r/2)((a-c)(A-C) + 4bB)/r,  r = eig-gap(S0)
Host ships 6 packed bf16 planes per core: (a+c), (a-c), 2b, A, 2B, C —
an invertible linear relabeling of each input tensor. All device tensor
ops run as packed-bf16 tensor_tensor (2x DVE mode); transcendentals on
ACT under the single Exp/Ln/Square table (sqrt = Exp(0.5*Ln)).

Sharding: pure data parallel over 8 cores; per-chunk partial sums of
L^2-parts and G-parts land in accumulator columns (DVE square +
tensor_reduce; tensor_tensor_reduce crashes the exec unit on real HW);
host combines with weights 0.5 / 2.0 and divides by B.
"""

import contextlib
import types

import numpy as np
import ml_dtypes

import bass_rust
import concourse.mybir as mybir
from concourse import bacc
from concourse.hw_specs import get_activation_tables
from concourse.tile import TileContext

N_CORES = 8
B_TOTAL = 1_048_576
P = 128
SHARD = B_TOTAL // N_CORES   # 131072 rows per core
CPT = SHARD // P             # 1024 rows per partition
NT = (448, 576)              # chunk widths (cols per chunk)

F32 = mybir.dt.float32
BF16 = mybir.dt.bfloat16
OP = mybir.AluOpType
AF = mybir.ActivationFunctionType

LN_QUARTER = float(np.log(0.25))

# engine per op: V = DVE, G = Pool, S = ACT (fixed for transcendentals)
ASSIGN = {
    "r2": "V",
    "SA": "G", "DA": "V", "z1": "V", "z2q": "G", "d4": "V",
    "x1": "V", "x2": "V", "N2": "V", "w": "V",
    "u1": "V", "u2": "V", "S2": "V",
    "L": "V", "k": "V", "kk": "V", "kkm": "V", "argB": "V",
}

PLANES = ("df", "b2", "A", "B2", "Bp", "C", "tp")
# sub-DMA plane groups: critical head planes land first
DMA_GROUPS = ((0, 2), (2, 6), (6, 7))



def _patched_insert_act_table_loads(self):
    """Force the single Exp/Ln/Square table so the greedy per-instruction
    set selection can't thrash (each reload costs ~1.3us)."""
    has_activation = any(
        isinstance(i, mybir.InstActivation)
        for b in self.main_func.blocks
        for i in b.instructions
    )
    if not has_activation:
        return
    need = {AF.Exp, AF.Ln, AF.Square}
    tables = [
        (name, funcs if need <= funcs else set())
        for name, funcs in get_activation_tables(self.m.arch).items()
    ]
    bass_rust.insert_act_table_loads(self, tables)



ACT_OPS = {"lr", "R", "rinv", "F1", "F2", "LD", "dinvh", "lnB", "sqB",
           "G2B"}
ASSIGN_NAMES = {"t4", "t3", "r2", "SA", "DA", "z1", "z2q", "d4", "x1", "x2",
                "N2", "w", "u1", "u2", "P1", "P2", "S2", "L", "k", "kk",
                "kkm", "argB", "lr", "R", "rinv", "F1", "F2", "LD", "dinvh",
                "lnB", "sqB", "G2B", "ttrL", "ttrG"}
TAIL_SPLIT = {"k", "kk", "kkm", "lnB", "sqB", "argB", "G2B", "ttrG"}


def _op_cost(name, eng, cc):
    if eng == "D":
        if name.startswith("store"):
            return 60.0
        return 0.7111 * cc
    if eng == "S":
        return 0.8333 * cc + 185
    if eng == "G":
        return 2.0833 * cc + 95
    if name.startswith("ttr"):
        return 1.0417 * cc + 60
    if name.startswith("sq"):
        return 0.5208 * cc + 60
    if name.startswith("kkm"):
        return 0.2604 * cc + 60
    return 0.5208 * cc + 60


def _list_schedule(nodes):
    """nodes: dict key -> (deps, engine, cost). Greedy earliest-start with
    downstream-critical-path tie-break, modeling: in-order engines, SP DMA
    issue serialization, DMA transfer serialization, sem latencies, ACT
    pipeline (write-ack) delay."""
    cp = {}
    succs = {k: [] for k in nodes}
    for k, (deps, eng, c) in nodes.items():
        for d in deps:
            succs[d].append(k)

    order_keys = list(nodes)

    def get_cp(k):
        if k in cp:
            return cp[k]
        cp[k] = nodes[k][2] + max((get_cp(s) for s in succs[k]), default=0.0)
        return cp[k]

    import sys as _s
    _s.setrecursionlimit(100000)
    for k in order_keys:
        get_cp(k)
    finish = {}      # when consumers may start (incl. sem latency at prod side)
    avail = {"V": 0.0, "S": 2390.0, "G": 0.0, "D": 0.0, "SP": 600.0}
    done = set()
    order = []
    pending = set(order_keys)
    while pending:
        best = None
        for k in pending:
            deps, eng, c = nodes[k]
            if any(d not in done for d in deps):
                continue
            ready = 0.0
            for d in deps:
                lat = 0.0
                deng = nodes[d][1]
                if deng != eng:
                    lat = 550.0 if deng == "D" else 150.0
                ready = max(ready, finish[d] + lat)
            if eng == "D":
                issue = avail["SP"] + 650.0
                start = max(avail["D"], issue + 780.0, ready)
            else:
                start = max(avail[eng], ready)
            key = (start, -cp[k])
            if best is None or key < best[0]:
                best = (key, k, eng, start, c)
        key, k, eng, start, c = best
        f = start + c
        if eng == "D":
            avail["SP"] += 650.0
            avail["D"] = f
        else:
            avail[eng] = f
        if eng == "S":
            f += 185.0  # ACT write-ack pipeline before sem fires
        finish[k] = f
        done.add(k)
        pending.discard(k)
        order.append(k)
    return order


def _register_const(nc, value, dtype=F32):
    if (dtype, value) in nc.const_aps.aps:
        return
    t = nc.alloc_sbuf_tensor(f"const-{dtype.name}-{value}", [128, 1], dtype)
    nc.gpsimd.memset(t.ap(), value)
    nc.const_aps.aps[(dtype, value)] = t.ap()


def build(nt=NT, assign=None, order="pipe", node_order=None, record=None):
    assign = dict(ASSIGN if assign is None else assign)
    chunks = (CPT // nt,) * nt if isinstance(nt, int) else tuple(nt)
    assert sum(chunks) == CPT
    nt = len(chunks)
    nc = bacc.Bacc()
    nc.insert_act_table_loads = types.MethodType(_patched_insert_act_table_loads, nc)
    _register_const(nc, 1e-30)
    _register_const(nc, LN_QUARTER)
    nc.multi_engine_barrier([mybir.EngineType.Pool, mybir.EngineType.Activation])
    NP = len(PLANES)
    inp = nc.dram_tensor("inp", [P, NP * CPT], BF16, kind="ExternalInput")
    out = nc.dram_tensor("out", [P, 4 * nt], F32, kind="ExternalOutput")

    with TileContext(nc) as tc, contextlib.ExitStack() as stack:
        iops = [stack.enter_context(tc.tile_pool(name=f"io{t}", bufs=1))
                for t in range(nt)]
        wps = [stack.enter_context(tc.tile_pool(name=f"work{t}", bufs=1))
               for t in range(nt)]
        V, S, G = nc.vector, nc.scalar, nc.gpsimd

        def make_chunk_nodes(t, coff, cc, nsplit):
            """Emit-closures + dep/eng/cost metadata for chunk t.
            Returns dict key->(emit_fn, deps, eng, cost)."""
            iop, wp = iops[t], wps[t]
            tiles = {}
            nodes = {}

            def K(n):
                return (t, n)

            def add(name, fn, deps, eng, cost_cc):
                nodes[K(name)] = (fn, tuple(K(d) for d in deps), eng,
                                  _op_cost(name, eng, cost_cc))

            def wt(name, dt=BF16):
                if name not in tiles:
                    tiles[name] = wp.tile([P, cc], dt, tag=name,
                                          name=f"{name}{t}")
                return tiles[name]

            def eng_of(name):
                base = name
                if base not in ASSIGN_NAMES and base[:-1] in ASSIGN_NAMES:
                    base = base[:-1]
                if base in ACT_OPS:
                    return "S"
                return assign.get((t, base), assign.get(base, "V"))

            def tt(name, n0, n1, op, sl=None, base=None):
                eng = eng_of(name if base is None else base)

                def f():
                    o = wt(base or name)
                    i0, i1 = tiles[n0], tiles[n1]
                    if sl is not None:
                        o, i0, i1 = o[:, sl], i0[:, sl], i1[:, sl]
                    e = V if eng == "V" else G
                    return e.tensor_tensor(o, i0, i1, op)
                return f, eng

            def act(name, n0, func, scale=1.0, bias=0.0, sl=None, base=None):
                def f():
                    o = wt(base or name)
                    i0 = tiles[n0]
                    if sl is not None:
                        o, i0 = o[:, sl], i0[:, sl]
                    return S.activation(o, i0, func, scale=scale, bias=bias)
                return f, "S"

            # loads — spread issue across SP/ACT/DVE DGE queues
            def load(gi):
                def f():
                    if "in" not in tiles:
                        tiles["in"] = iop.tile([P, NP * cc], BF16, tag="in",
                                               name=f"in{t}")
                        for i, n in enumerate(PLANES):
                            tiles[n] = tiles["in"][:, i * cc:(i + 1) * cc]
                    it = tiles["in"]
                    base_c = NP * coff
                    g0, g1 = DMA_GROUPS[gi]
                    eng = nc.sync
                    return eng.dma_start(
                        out=it[:, g0 * cc:g1 * cc],
                        in_=inp[:, base_c + g0 * cc:base_c + g1 * cc])
                return f

            for gi, (g0, g1) in enumerate(DMA_GROUPS):
                nodes[K(f"load{gi}")] = (load(gi), (), "D",
                                         _op_cost("load", "D",
                                                  (g1 - g0) * cc))

            def simple(name, mk, deps, cost_cc=None):
                fn, eng = mk
                add(name, fn, deps, eng, cost_cc or cc)

            def t43f():
                o = wt("t43")
                dfb2 = tiles["in"][:, 0:2 * cc]
                return V.tensor_tensor(o, dfb2, dfb2, OP.mult)
            def wt2(name):
                if name not in tiles or tiles[name] is None:
                    tiles[name] = wp.tile([P, 2 * cc], BF16, tag=name,
                                          name=f"{name}{t}")
                return tiles[name]

            def t43f2():
                o = wt2("t43")
                dfb2 = tiles["in"][:, 0:2 * cc]
                return V.tensor_tensor(o, dfb2, dfb2, OP.mult)
            add("t43", t43f2, ["load0"], "V", 2 * cc)

            def r2f():
                o = wt("r2")
                t43 = tiles["t43"]
                return V.tensor_tensor(o, t43[:, :cc], t43[:, cc:], OP.add)
            add("r2", r2f, ["t43"], eng_of("r2"), cc)
            simple("lr", act("lr", "r2", AF.Ln, bias=1e-30), ["r2"])
            simple("R", act("R", "lr", AF.Exp, scale=0.5), ["lr"])
            simple("rinv", act("rinv", "lr", AF.Exp, scale=-0.5), ["lr"])
            def actF(scale, half):
                def f():
                    F = wt2("F12")
                    sl = slice(0, cc) if half == 0 else slice(cc, 2 * cc)
                    return S.activation(F[:, sl], tiles["R"], AF.Exp,
                                        scale=scale)
                return f
            add("F1", actF(0.5, 0), ["R"], "S", cc)
            add("F2", actF(-0.5, 1), ["R"], "S", cc)
            simple("SA", tt("SA", "A", "C", OP.add), ["load1"])
            simple("DA", tt("DA", "A", "C", OP.subtract), ["load1"])
            simple("z1", tt("z1", "A", "C", OP.mult), ["load1"])
            simple("z2q", tt("z2q", "Bp", "Bp", OP.mult), ["load1"])
            simple("d4", tt("d4", "z1", "z2q", OP.subtract), ["z1", "z2q"])
            simple("LD", act("LD", "d4", AF.Ln, bias=1e-30), ["d4"])
            simple("dinvh", act("dinvh", "LD", AF.Exp, scale=-0.5,
                                bias=LN_QUARTER), ["LD"])
            simple("L", tt("L", "LD", "tp", OP.subtract), ["LD", "load2"])
            simple("x1", tt("x1", "df", "DA", OP.mult), ["load0", "DA"])
            simple("x2", tt("x2", "B2", "b2", OP.mult), ["load0", "load1"])
            simple("N2", tt("N2", "x1", "x2", OP.add), ["x1", "x2"])
            simple("w", tt("w", "N2", "rinv", OP.mult), ["N2", "rinv"])
            def uf(op, half):
                def f():
                    U = wt2("U12")
                    sl = slice(0, cc) if half == 0 else slice(cc, 2 * cc)
                    e = V if eng_of("u1" if half == 0 else "u2") == "V" else G
                    return e.tensor_tensor(U[:, sl], tiles["SA"], tiles["w"],
                                           op)
                return f
            add("u1", uf(OP.subtract, 0), ["SA", "w"], eng_of("u1"), cc)
            add("u2", uf(OP.add, 1), ["SA", "w"], eng_of("u2"), cc)

            def p12f():
                o = wt2("P12")
                return V.tensor_tensor(o, tiles["U12"], tiles["F12"],
                                       OP.mult)
            add("P12", p12f, ["u1", "u2", "F1", "F2"], "V", 2 * cc)

            def s2f():
                o = wt("S2")
                p = tiles["P12"]
                return V.tensor_tensor(o, p[:, :cc], p[:, cc:], OP.add)
            add("S2", s2f, ["P12"], eng_of("S2"), cc)

            # accumulator tile: [L, G half cols...]
            cacc = wp.tile([P, 1 + nsplit], F32, tag="cacc", name=f"cacc{t}")

            def sqf(n0, col, sl=None):
                def f():
                    sq = wt(f"sq{col}")
                    i0 = tiles[n0]
                    if sl is not None:
                        sq_, i0_ = sq[:, sl], i0[:, sl]
                    else:
                        sq_, i0_ = sq[:], i0[:]
                    return V.tensor_tensor(sq_, i0_, i0_, OP.mult)
                return f

            def redf(col, sl=None):
                def f():
                    sq = tiles[f"sq{col}"]
                    i0_ = sq[:] if sl is None else sq[:, sl]
                    return V.tensor_reduce(cacc[:, col:col + 1], i0_,
                                           mybir.AxisListType.X, OP.add)
                return f

            if assign.get("_accL") == "S":
                def aLf():
                    return S.activation(wt("sq0"), tiles["L"], AF.Square,
                                        accum_out=cacc[:, 0:1])
                add("ttrL", aLf, ["L"], "S", cc)
            else:
                add("sqL", sqf("L", 0), ["L"], "V", cc)
                add("ttrL", redf(0), ["sqL"], "V", cc)


            # tail: split into nsplit column ranges
            bounds = [cc * i // nsplit for i in range(nsplit + 1)]
            for h in range(nsplit):
                sl = slice(bounds[h], bounds[h + 1])
                hc = bounds[h + 1] - bounds[h]
                sfx = str(h)

                def hsimple(name, mk, deps):
                    fn, eng = mk
                    add(name + sfx, fn, deps, eng, hc)

                hsimple("k", tt("k", "S2", "dinvh", OP.mult, sl=sl, base="k"),
                        ["S2", "dinvh"])
                hsimple("kk", tt("kk", "k", "k", OP.mult, sl=sl, base="kk"),
                        [f"k{h}"])

                def kkmf(sl=sl):
                    def f():
                        o = wt("kkm")
                        return V.tensor_scalar(o[:, sl], tiles["kk"][:, sl],
                                               1.0, 0.0, OP.subtract, OP.max)
                    return f
                add("kkm" + sfx, kkmf(), [f"kk{h}"], eng_of("kkm"), hc)
                hsimple("lnB", act("lnB", "kkm", AF.Ln, bias=1e-30, sl=sl,
                                   base="lnB"), [f"kkm{h}"])
                hsimple("sqB", act("sqB", "lnB", AF.Exp, scale=0.5, sl=sl,
                                   base="sqB"), [f"lnB{h}"])
                hsimple("argB", tt("argB", "k", "sqB", OP.add, sl=sl,
                                   base="argB"), [f"k{h}", f"sqB{h}"])
                hsimple("G2B", act("G2B", "argB", AF.Ln, sl=sl, base="G2B"),
                        [f"argB{h}"])
                add("sqG" + sfx, sqf("G2B", 1 + h, sl=sl), [f"G2B{h}"],
                    "V", hc)
                add("ttrG" + sfx, redf(1 + h, sl=sl), [f"sqG{h}"], "V", hc)

            def storef():
                return nc.sync.dma_start(
                    out=out[:, 4 * t:4 * t + 1 + nsplit], in_=cacc[:])
            add("store", storef,
                ["ttrL"] + [f"ttrG{h}" for h in range(nsplit)], "D", 1)
            return nodes

        nsplits = assign.get("_nsplits")
        if nsplits is None:
            nsplits = [1] * (nt - 1) + [2]
        all_nodes = {}
        coff = 0
        for t in range(nt):
            all_nodes.update(make_chunk_nodes(t, coff, chunks[t],
                                              nsplits[t]))
            coff += chunks[t]

        meta = {k: (v[1], v[2], v[3]) for k, v in all_nodes.items()}
        if node_order is None:
            node_order = _list_schedule(meta)
        else:
            node_order = [tuple(k) for k in node_order]
            miss = set(all_nodes) - set(node_order)
            assert not miss, f"node_order missing {miss}"
        for k in node_order:
            ret = all_nodes[k][0]()
            if record is not None and ret is not None:
                nm = getattr(ret, "name", None)
                if nm is None and hasattr(ret, "ins"):
                    nm = getattr(ret.ins, "name", None)
                if nm is not None:
                    record.setdefault(k, []).append(nm)

    nc.finalize()
    return nc


_CACHED = {}


def _key(nt, assign, order="pipe"):
    return (nt if isinstance(nt, int) else tuple(nt),
            None if assign is None else tuple(sorted(assign.items())), order)


def _get_nc(nt=NT, assign=None, order="pipe"):
    key = _key(nt, assign, order)
    if key not in _CACHED:
        _CACHED[key] = build(nt, assign, order)
    return _CACHED[key]


class _Runner:
    """Cached-jit SPMD runner (shard_map over 8 cores)."""

    def __init__(self, nt=NT, assign=None, order="pipe"):
        import jax
        from jax.sharding import Mesh, PartitionSpec
        from jax.experimental.shard_map import shard_map
        from concourse import bass2jax

        self.jax = jax
        nc = _get_nc(nt, assign, order)
        self.nc = nc
        bass2jax.install_neuronx_cc_hook()

        partition_name = (nc.partition_id_tensor.name
                          if nc.partition_id_tensor else None)
        in_names, out_names, out_avals, zero_outs = [], [], [], []
        for alloc in nc.m.functions[0].allocations:
            if not isinstance(alloc, mybir.MemoryLocationSet):
                continue
            name = alloc.memorylocations[0].name
            if alloc.kind == "ExternalInput":
                if name != partition_name:
                    in_names.append(name)
            elif alloc.kind == "ExternalOutput":
                shape = tuple(alloc.tensor_shape)
                dtype = mybir.dt.np(alloc.dtype)
                out_names.append(name)
                out_avals.append(jax.core.ShapedArray(shape, dtype))
                zero_outs.append(np.zeros(shape, dtype))
        self.in_names = list(in_names)
        self.out_names = out_names
        self.zero_outs = zero_outs
        n_params = len(in_names)
        n_outs = len(out_avals)
        all_names = in_names + out_names
        if partition_name is not None:
            all_names.append(partition_name)

        def _body(*args):
            operands = list(args)
            if partition_name is not None:
                operands.append(bass2jax.partition_id_tensor())
            outs = bass2jax._bass_exec_p.bind(
                *operands,
                out_avals=tuple(out_avals),
                in_names=tuple(all_names),
                out_names=tuple(out_names),
                lowering_input_output_aliases=(),
                sim_require_finite=True,
                sim_require_nnan=True,
                nc=nc,
            )
            return tuple(outs)

        devices = jax.devices()[:N_CORES]
        mesh = Mesh(np.asarray(devices), ("core",))
        in_specs = (PartitionSpec("core"),) * (n_params + n_outs)
        out_specs = (PartitionSpec("core"),) * n_outs
        donate = tuple(range(n_params, n_params + n_outs))
        self.sharded = jax.jit(
            shard_map(_body, mesh=mesh, in_specs=in_specs,
                      out_specs=out_specs, check_rep=False),
            donate_argnums=donate, keep_unused=True,
        )
        self.n_params = n_params
        self.n_outs = n_outs

    def concat_inputs(self, in_maps):
        return [
            np.concatenate([np.asarray(m[name]) for m in in_maps], axis=0)
            for name in self.in_names
        ]

    def call_raw(self, concat_in):
        zeros = [np.zeros((N_CORES * z.shape[0], *z.shape[1:]), z.dtype)
                 for z in self.zero_outs]
        return self.sharded(*concat_in, *zeros)

    def __call__(self, in_maps):
        out_arrs = self.call_raw(self.concat_inputs(in_maps))
        per_core_rows = self.zero_outs[0].shape[0]
        out0 = np.asarray(out_arrs[0]).reshape(N_CORES, per_core_rows, -1)
        return [{self.out_names[0]: out0[c]} for c in range(N_CORES)]


_RUNNERS = {}


def _get_runner(nt=NT, assign=None, order="pipe"):
    key = _key(nt, assign, order)
    if key not in _RUNNERS:
        _RUNNERS[key] = _Runner(nt, assign, order)
    return _RUNNERS[key]


def _chunks(nt=NT):
    return (CPT // nt,) * nt if isinstance(nt, int) else tuple(nt)


def _in_maps(prediction, actual, nt=NT):
    """Host-side plane construction: invertible linear relabeling per
    tensor, cast to bf16, packed per-chunk [P, 6*cc] blocks."""
    BF = ml_dtypes.bfloat16
    chunks = _chunks(nt)
    pred = np.ascontiguousarray(prediction, dtype=np.float32)
    act = np.ascontiguousarray(actual, dtype=np.float32)
    maps = []
    for c in range(N_CORES):
        ps = pred[c * SHARD:(c + 1) * SHARD].reshape(P, CPT, 3)
        as_ = act[c * SHARD:(c + 1) * SHARD].reshape(P, CPT, 3)
        a0, b0, c0 = ps[:, :, 0], ps[:, :, 1], ps[:, :, 2]
        A, B, C = as_[:, :, 0], as_[:, :, 1], as_[:, :, 2]
        planes = {
            "tp": (a0 + c0), "df": (a0 - c0), "b2": (2.0 * b0),
            "A": A, "B2": (2.0 * B), "Bp": B, "C": C,
        }
        buf = np.empty((P, len(PLANES) * CPT), dtype=BF)
        off = 0
        for cc in chunks:
            for i, n in enumerate(PLANES):
                seg = planes[n][:, off:off + cc]
                buf[:, len(PLANES) * off + i * cc:
                    len(PLANES) * off + (i + 1) * cc] = seg.astype(BF)
            off += cc
        maps.append({"inp": buf})
    return maps


def run(prediction, actual, nt=NT, assign=None, order="pipe"):
    runner = _get_runner(nt, assign, order)
    results = runner(_in_maps(prediction, actual, nt))
    lsum = 0.0
    gsum = 0.0
    for om in results:
        o = om["out"].astype(np.float64)   # [P, 4*nt]
        lsum += o[:, 0::4].sum()
        gsum += (o[:, 1::4].sum() + o[:, 2::4].sum() + o[:, 3::4].sum())
    total = 0.5 * lsum + 2.0 * gsum
    return np.float32(total / B_TOTAL), results


def kernel(prediction, actual):
    value, _ = run(prediction, actual)
    return value
